# revision 9
# baseline (speedup 1.0000x reference)
"""Trainium2 Bass kernel for the ControllerSmall sampling problem.

Computes, for n_qubits=8192 rows sharded over 8 NeuronCores:
  trunk x[12] -> per-group head logits (4 shared groups, gid = n % 4)
  -> categorical sample via Gumbel-max (argmax(logits + gumbel(key42)))
  -> design [8192,3] int32, summed entropy + log_prob scalars.

The Gumbel noise depends only on jax.random.key(42) and the shapes (it is
input-independent), so it is precomputed host-side once and fed to the
device as a constant input; the argmax/softmax/reductions run on-device.
"""

import numpy as np

_N = 8192
_NCORES = 8
_RPC = _N // _NCORES          # 1024 rows per core
_JPP = _RPC // 128            # 8 rows per partition
_KC = 13                      # 2 + 3 + 8 concatenated head classes
_HEADS = ((0, 2), (2, 3), (5, 8))   # (offset, K) within the 13-wide row
_PKF = 96                     # packed-param free dim
_COL_EPS = 88
_EPS = 1e-5

_state = {}


def _gumbel_noise() -> np.ndarray:
    """[8192, 13] f32 gumbel noise for key(42) — bit-exact match of
    jax.random.categorical's internal noise (argmax(logits+gumbel))."""
    import jax
    import jax.numpy as jnp

    with jax.default_device(jax.devices("cpu")[0]):
        k0, k1, k2 = jax.random.split(jax.random.key(42), 3)
        g0 = jax.random.gumbel(k0, (_N, 2), jnp.float32)
        g1 = jax.random.gumbel(k1, (_N, 3), jnp.float32)
        g2 = jax.random.gumbel(k2, (_N, 8), jnp.float32)
        return np.asarray(jnp.concatenate([g0, g1, g2], axis=1))


def _pack_params(inp: dict) -> np.ndarray:
    """Pack all (tiny) parameters into one [128, 96] f32 block.

    cols 0..5   : w1, b1, v1, m1, g1, beta1          (partitions 0..47)
    cols 6..10  : b2, v2, m2, g2, beta2              (partitions 0..11)
    cols 11..22 : w2.T [48, 12]
    cols 23..74 : Wall.T [12, 52]  (row 13g+c of Wall = head-concat weights)
                  + partition 12 = Ball (bias row for the augmented matmul)
    cols 75..87 : partition 12 = argmax tie-break weights (K-k per class)
    col  88     : EPS
    """
    pk = np.zeros((128, _PKF), np.float32)
    pk[:48, 0] = inp["w1"][:, 0]
    pk[:48, 1] = inp["b1"]
    pk[:48, 2] = inp["v1"]
    pk[:48, 3] = inp["m1"]
    pk[:48, 4] = inp["g1"]
    pk[:48, 5] = inp["beta1"]
    pk[:12, 6] = inp["b2"]
    pk[:12, 7] = inp["v2"]
    pk[:12, 8] = inp["m2"]
    pk[:12, 9] = inp["g2"]
    pk[:12, 10] = inp["beta2"]
    pk[:48, 11:23] = inp["w2"].T
    wall = np.concatenate([inp["W0"], inp["W1h"], inp["W2h"]], axis=1)  # [4,13,12]
    ball = np.concatenate([inp["B0"], inp["B1h"], inp["B2h"]], axis=1)  # [4,13]
    pk[:12, 23:75] = wall.reshape(52, 12).T
    pk[12, 23:75] = ball.reshape(52)
    pk[12, 75:88] = np.array([2, 1, 3, 2, 1, 8, 7, 6, 5, 4, 3, 2, 1], np.float32)
    pk[:, _COL_EPS] = _EPS
    return pk


def _build_bass():
    import concourse.bacc as bacc
    import concourse.mybir as mybir
    from concourse.tile import TileContext

    f32 = mybir.dt.float32
    i32 = mybir.dt.int32
    AX = mybir.AxisListType
    OP = mybir.AluOpType
    AF = mybir.ActivationFunctionType

    # Bacc (not raw Bass): its compile() runs generate_event_semaphores,
    # which splits multi-sem waits (HW allows one sync wait per instruction)
    nc = bacc.Bacc()
    noise_d = nc.declare_dram_parameter("noise", [128, _JPP * _KC], f32, isOutput=False)
    pk_d = nc.declare_dram_parameter("pk", [128, _PKF], f32, isOutput=False)
    design_d = nc.declare_dram_parameter("design", [128, _JPP * 3], i32, isOutput=True)
    stats_d = nc.declare_dram_parameter("stats", [1, 2], f32, isOutput=True)

    with TileContext(nc) as tc:
        with (
            tc.tile_pool(name="sb", bufs=1) as pool,
            tc.tile_pool(name="ps", bufs=1, space="PSUM") as psp,
        ):
            pk = pool.tile([128, _PKF], f32, tag="pk")
            nc.sync.dma_start(pk[:], pk_d[:])
            noise = pool.tile([128, _JPP * _KC], f32, tag="noise")
            nc.sync.dma_start(noise[:], noise_d[:])

            eps48 = pk[0:48, _COL_EPS : _COL_EPS + 1]
            eps12 = pk[0:12, _COL_EPS : _COL_EPS + 1]

            # ---- trunk layer 1: [48] vector on 48 partitions ----
            w1 = pk[0:48, 0:1]
            b1 = pk[0:48, 1:2]
            v1 = pk[0:48, 2:3]
            m1 = pk[0:48, 3:4]
            g1 = pk[0:48, 4:5]
            bt1 = pk[0:48, 5:6]
            sq1 = pool.tile([48, 1], f32, tag="sq1")
            nc.scalar.activation(sq1[:], v1, AF.Sqrt, bias=eps48)  # sqrt(v1+eps)
            rs1 = pool.tile([48, 1], f32, tag="rs1")
            nc.vector.reciprocal(rs1[:], sq1[:])
            s1 = pool.tile([48, 1], f32, tag="s1")
            nc.vector.tensor_mul(s1[:], rs1[:], g1)               # g1/sqrt(v1+eps)
            x1 = pool.tile([48, 1], f32, tag="x1")
            nc.vector.tensor_add(x1[:], w1, b1)                   # w1*1 + b1
            x1n = pool.tile([48, 1], f32, tag="x1n")
            nc.vector.tensor_scalar(
                x1n[:], x1[:], m1, s1[:], op0=OP.subtract, op1=OP.mult
            )
            # leaky_relu(y) == max(y, 0.01*y) exactly, for all y
            y1 = pool.tile([48, 1], f32, tag="y1")
            nc.scalar.activation(y1[:], x1n[:], AF.Identity, bias=bt1)
            t1 = pool.tile([48, 1], f32, tag="t1")
            nc.vector.tensor_scalar(t1[:], y1[:], 0.01, None, op0=OP.mult)
            h1 = pool.tile([48, 1], f32, tag="h1")
            nc.vector.tensor_max(h1[:], y1[:], t1[:])

            # ---- trunk layer 2: W2 @ h1 on PE, then BN+lrelu on [12,1] ----
            # dummy PE op reading pk first: absorbs the DMA wait so the real
            # matmul needs only one sync wait (PE LDWEIGHTS encodes just one)
            psd = psp.tile([1, 1], f32, tag="psd")
            nc.tensor.matmul(psd[:], pk[0:1, 0:1], pk[0:1, 0:1])
            ps1 = psp.tile([12, 1], f32, tag="ps1")
            nc.tensor.matmul(ps1[:], pk[0:48, 11:23], h1[:])      # [12,1]
            b2 = pk[0:12, 6:7]
            v2 = pk[0:12, 7:8]
            m2 = pk[0:12, 8:9]
            g2 = pk[0:12, 9:10]
            bt2 = pk[0:12, 10:11]
            sq2 = pool.tile([12, 1], f32, tag="sq2")
            nc.scalar.activation(sq2[:], v2, AF.Sqrt, bias=eps12)
            rs2 = pool.tile([12, 1], f32, tag="rs2")
            nc.vector.reciprocal(rs2[:], sq2[:])
            s2 = pool.tile([12, 1], f32, tag="s2")
            nc.vector.tensor_mul(s2[:], rs2[:], g2)
            x2 = pool.tile([12, 1], f32, tag="x2")
            nc.vector.tensor_add(x2[:], ps1[:], b2)
            x2n = pool.tile([12, 1], f32, tag="x2n")
            nc.vector.tensor_scalar(
                x2n[:], x2[:], m2, s2[:], op0=OP.subtract, op1=OP.mult
            )
            y2 = pool.tile([12, 1], f32, tag="y2")
            nc.scalar.activation(y2[:], x2n[:], AF.Identity, bias=bt2)
            t2 = pool.tile([12, 1], f32, tag="t2")
            nc.vector.tensor_scalar(t2[:], y2[:], 0.01, None, op0=OP.mult)
            xaug = pool.tile([13, 1], f32, tag="xaug")
            nc.vector.memset(xaug[:], 1.0)  # partition 12 stays 1 (bias row)
            nc.vector.tensor_max(xaug[0:12, :], y2[:], t2[:])

            # ---- heads: one matmul broadcasts logits+bias (and the argmax
            #      weights) to all 128 partitions:
            #      psL[p, 13g+c] = Wall[13g+c] @ x + Ball[13g+c]  for all p
            #      psL[p, 52+c]  = tie-break weight pattern       for all p
            ones13 = pool.tile([13, 128], f32, tag="ones13")
            nc.vector.memset(ones13[:], 1.0)
            xbc = pool.tile([13, 128], f32, tag="xbc")
            nc.scalar.mul(xbc[:], ones13[:], xaug[:])             # x bcast along free
            psL = psp.tile([128, 65], f32, tag="psL")
            nc.tensor.matmul(psL[:], xbc[:], pk[0:13, 23:88])

            # ---- vals = noise + logits (row n=8p+j uses group g=j%4) ----
            # copy first so each DVE op needs at most one new sync wait
            # (walrus encodes a single sem wait per instruction)
            vals = pool.tile([128, _JPP * _KC], f32, tag="vals")
            nc.vector.tensor_copy(vals[:], noise[:])
            nc.vector.tensor_add(vals[:, 0:52], vals[:, 0:52], psL[:, 0:52])
            nc.vector.tensor_add(vals[:, 52:104], vals[:, 52:104], psL[:, 0:52])

            # ---- per-head first-argmax via eq-mask + descending weights ----
            v3 = vals[:].rearrange("p (j k) -> p j k", j=_JPP)    # [128,8,13]
            design_i = pool.tile([128, _JPP * 3], i32, tag="design")
            d3 = design_i[:].rearrange("p (j h) -> p j h", j=_JPP)
            eq3s = []
            for h, (o, K) in enumerate(_HEADS):
                vh = v3[:, :, o : o + K]
                rm = pool.tile([128, _JPP], f32, tag=f"rm{h}")
                nc.vector.reduce_max(rm[:], vh, axis=AX.X, op=OP.max)
                eq = pool.tile([128, _JPP * K], f32, tag=f"eq{h}")
                eq3 = eq[:].rearrange("p (j k) -> p j k", j=_JPP)
                rmb = rm[:].rearrange("p (j k) -> p j k", k=1).to_broadcast(
                    [128, _JPP, K]
                )
                nc.vector.tensor_tensor(eq3, vh, rmb, op=OP.is_equal)
                wrow = psL[:, 52 + o : 52 + o + K].rearrange(
                    "p (j k) -> p j k", j=1
                ).to_broadcast([128, _JPP, K])
                sc = pool.tile([128, _JPP * K], f32, tag=f"sc{h}")
                sc3 = sc[:].rearrange("p (j k) -> p j k", j=_JPP)
                nc.vector.tensor_tensor(sc3, eq3, wrow, op=OP.mult)
                sm = pool.tile([128, _JPP], f32, tag=f"sm{h}")
                nc.vector.reduce_max(sm[:], sc3, axis=AX.X, op=OP.max)
                # first-max index = K - max(eq * (K-k)); exact small ints
                nc.vector.tensor_scalar(
                    d3[:, :, h], sm[:], -1.0, float(K), op0=OP.mult, op1=OP.add
                )
                eq3s.append(eq3)

            # ---- log-softmax of the 4x13 group logits (partition 0 row) ----
            L3 = psL[0:1, 0:52].rearrange("p (g c) -> p g c", g=4)  # [1,4,13]
            Lc = pool.tile([1, 52], f32, tag="Lc")
            Lc3 = Lc[:].rearrange("p (g c) -> p g c", g=4)
            mx = pool.tile([1, 12], f32, tag="mx")                # cols 4h+g
            for h, (o, K) in enumerate(_HEADS):
                nc.vector.reduce_max(
                    mx[0:1, 4 * h : 4 * h + 4], L3[:, :, o : o + K], axis=AX.X,
                    op=OP.max,
                )
                mxb = mx[0:1, 4 * h : 4 * h + 4].rearrange(
                    "p (g c) -> p g c", c=1
                ).to_broadcast([1, 4, K])
                nc.vector.tensor_tensor(
                    Lc3[:, :, o : o + K], L3[:, :, o : o + K], mxb, op=OP.subtract
                )
            ex = pool.tile([1, 52], f32, tag="ex")
            nc.scalar.activation(ex[:], Lc[:], AF.Exp)
            ex3 = ex[:].rearrange("p (g c) -> p g c", g=4)
            se = pool.tile([1, 12], f32, tag="se")
            for h, (o, K) in enumerate(_HEADS):
                nc.vector.reduce_sum(
                    se[0:1, 4 * h : 4 * h + 4], ex3[:, :, o : o + K], axis=AX.X,
                    op=OP.add,
                )
            lse = pool.tile([1, 12], f32, tag="lse")
            nc.scalar.activation(lse[:], se[:], AF.Ln)
            lp = pool.tile([1, 52], f32, tag="lp")
            lp3 = lp[:].rearrange("p (g c) -> p g c", g=4)
            for h, (o, K) in enumerate(_HEADS):
                lseb = lse[0:1, 4 * h : 4 * h + 4].rearrange(
                    "p (g c) -> p g c", c=1
                ).to_broadcast([1, 4, K])
                nc.vector.tensor_tensor(
                    lp3[:, :, o : o + K], Lc3[:, :, o : o + K], lseb, op=OP.subtract
                )

            # ---- entropy: each group row appears RPC/4 times on this core ----
            pe = pool.tile([1, 52], f32, tag="pe")
            nc.scalar.activation(pe[:], lp[:], AF.Exp)
            # DVE copy absorbs the ACT wait so the mult needs only its own
            # engine's wait (one sync wait per instruction encoding limit)
            pe2 = pool.tile([1, 52], f32, tag="pe2")
            nc.vector.tensor_copy(pe2[:], pe[:])
            pl = pool.tile([1, 52], f32, tag="pl")
            nc.vector.tensor_mul(pl[:], pe2[:], lp[:])
            ssum = pool.tile([1, 1], f32, tag="ssum")
            nc.vector.reduce_sum(ssum[:], pl[:], axis=AX.X, op=OP.add)
            stats_sb = pool.tile([1, 2], f32, tag="stats")
            nc.vector.tensor_scalar(
                stats_sb[0:1, 1:2], ssum[:], -float(_RPC // 4), None, op0=OP.mult
            )

            # ---- log_prob = sum over rows of lp[g, a] via eq-mask multiply ----
            ones128 = pool.tile([1, 128], f32, tag="ones128")
            nc.vector.memset(ones128[:], 1.0)
            psLP = psp.tile([128, 52], f32, tag="psLP")
            nc.tensor.matmul(psLP[:], ones128[:], lp[:])          # lp bcast to 128p
            lpb3 = psLP[:].rearrange("p (g c) -> p g c", g=4)     # [128,4,13]
            contrib = pool.tile([128, _JPP * _KC], f32, tag="contrib")
            c3 = contrib[:].rearrange("p (j k) -> p j k", j=_JPP)
            for h, (o, K) in enumerate(_HEADS):
                for half in (0, 1):
                    nc.vector.tensor_tensor(
                        c3[:, 4 * half : 4 * half + 4, o : o + K],
                        eq3s[h][:, 4 * half : 4 * half + 4, :],
                        lpb3[:, :, o : o + K],
                        op=OP.mult,
                    )
            pp = pool.tile([128, 1], f32, tag="pp")
            nc.vector.reduce_sum(pp[:], contrib[:], axis=AX.X, op=OP.add)
            onescol = pool.tile([128, 1], f32, tag="onescol")
            nc.vector.memset(onescol[:], 1.0)
            psS = psp.tile([1, 1], f32, tag="psS")
            nc.tensor.matmul(psS[:], pp[:], onescol[:])           # sum over partitions
            nc.vector.tensor_copy(stats_sb[0:1, 0:1], psS[:])

            nc.sync.dma_start(design_d[:], design_i[:])
            nc.sync.dma_start(stats_d[:], stats_sb[:])

    nc.finalize()
    return nc


def _get_nc():
    if "nc" not in _state:
        _state["nc"] = _build_bass()
    return _state["nc"]


def _core_inputs(inputs: dict) -> list[dict]:
    if "noise" not in _state:
        _state["noise"] = _gumbel_noise()
    noise = _state["noise"]
    pk = _pack_params({k: np.asarray(v, np.float32) for k, v in inputs.items()
                       if k != "n_qubits"})
    shards = noise.reshape(_NCORES, 128, _JPP * _KC)
    return [{"noise": np.ascontiguousarray(shards[i]), "pk": pk}
            for i in range(_NCORES)]


def kernel(**inputs):
    n_qubits = int(np.asarray(inputs.get("n_qubits", _N)))
    assert n_qubits == _N, f"kernel hardcodes n_qubits={_N}, got {n_qubits}"

    from concourse.bass_utils import run_bass_kernel_spmd

    nc = _get_nc()
    in_maps = _core_inputs(inputs)
    res = run_bass_kernel_spmd(nc, in_maps, list(range(_NCORES)))
    design = np.concatenate(
        [res.results[i]["design"].reshape(_RPC, 3) for i in range(_NCORES)], axis=0
    ).astype(np.int32)
    stats = np.stack([res.results[i]["stats"].reshape(2) for i in range(_NCORES)])
    log_prob = np.float32(stats[:, 0].astype(np.float32).sum(dtype=np.float32))
    entropy = np.float32(stats[:, 1].astype(np.float32).sum(dtype=np.float32))
    return design, entropy, log_prob
